# revision 24
# baseline (speedup 1.0000x reference)
"""LongFormer sliding-window attention on 8 Trainium2 NeuronCores.

Sharding: batch*heads data-parallel. 24 (batch, head) pairs -> 8 cores,
each core owns one batch (core//4) and 3 consecutive heads (3*(core%4)).
No collectives.

v2 design (cost-model driven):
  - Projections run as fp8e4 DoubleRow matmuls (0.5 cyc/row) over a
    CONCATENATED contraction [x8 ; x_residual8] x [W8 ; W_residual8]:
    error-compensated fp8 => product of (x8+xr8)(W8+Wr8), ~0.1% error,
    at 1/2 the streamed rows of bf16 and 1/4 of baseline fp32r.
  - Q/K written to SBUF as fp16 [dims, seq] (3 groups of 128 partitions:
    q01 | k01 | q2k2); head-2's K is realigned to partitions 0:63 by a
    small SBUF->SBUF DMA so scores matmuls keep matching base partitions.
  - Scores (transposed, [key, query]) per 256-query chunk go into ONE
    3-bank PSUM tile, tightly packed (1280 cols interior); one bracket
    (start..stop) per 2KB bank so the lazy PSUM zeroing stays correct.
  - ONE exp activation per chunk-head over the whole packed score strip
    (ACT cost is per free-column + fixed overhead, so merging exps is the
    main ACT win; ACT is the end-state bottleneck engine).
  - Triangle masks: two strided pair-multiplies (fp16, 2x DVE mode).
  - PV in fp16, 65-wide (64 dims + ones column for the denominator),
    the 3 heads accumulate into one PSUM bank per query half.
  - Normalization: batched reciprocal [128,3] + one broadcast
    tensor-multiply into the fp16 output tile.
  - Projection stripes are interleaved between attention chunks so PE
    stripe work fills the gaps while ACT (exp) is the bottleneck.
"""

import os
import sys

import numpy as np
import ml_dtypes

sys.path.insert(0, "/opt/trn_rl_repo")

# debug bisect flags (default = fully optimized)
DBG_NO_DR = bool(int(os.environ.get("KDBG_NO_DR", "0")))
DBG_SPLIT_EXP = bool(int(os.environ.get("KDBG_SPLIT_EXP", "0")))
DBG_PLAIN_MASK = bool(int(os.environ.get("KDBG_PLAIN_MASK", "0")))
DBG_PLAIN_NORM = bool(int(os.environ.get("KDBG_PLAIN_NORM", "0")))

import concourse.bass as bass  # noqa: E402
import concourse.tile as tile  # noqa: E402
from concourse import bacc, mybir  # noqa: E402
from concourse import bass_utils  # noqa: E402

B, S, E = 2, 4096, 768
H, D = 12, 64
C = 16              # chunks of 256 queries
HPC = 3             # heads per core
N_CORES = 8
NT = 8              # 8 stripes of 512 tokens
KT2 = 12            # 6 kt blocks of x8 + 6 of x_residual8
VW = 195            # 3 heads x (64 dims + ones col)
WSCALE = 16.0       # weights quantized as fp8(16*W); fixed up downstream

# 3-term scaled-compensated fp8 product: 16*x@W ~= x8@w16 + x8@rw16 + rx8@w16
# expressed as 9 DoubleRow pairs over the stored blocks
# [x8(pairs 0-2) | rx8(pairs 3-5)] and [w16(pairs 0-2) | rw16(pairs 3-5)].
PAIR_TERMS = [(0, 0), (1, 1), (2, 2),   # x8 @ w16
              (3, 0), (4, 1), (5, 2),   # x8 @ rw16
              (0, 3), (1, 4), (2, 5)]   # rx8 @ w16  (w-pair, x-pair)

f32 = mybir.dt.float32
f16 = mybir.dt.float16
f8 = mybir.dt.float8e4
DR = mybir.MatmulPerfMode.DoubleRow
FP8 = ml_dtypes.float8_e4m3fn


def _chunk_blocks(c):
    """Packed score layout for chunk c: list of (t, base_col, qlo, qn).

    t indexes the relative key tile (kt_abs = 2*(c-1)+t); base_col is the
    column of the block inside the packed PSUM strip; [qlo, qlo+qn) is the
    query span (relative to the chunk) the block covers.
    """
    if c == 0:
        return [(2, 0, 0, 256), (3, 256, 0, 256), (4, 512, 0, 256),
                (5, 768, 128, 128)]
    if c == C - 1:
        return [(0, 0, 0, 128), (1, 128, 0, 256), (2, 384, 0, 256),
                (3, 640, 0, 256)]
    return [(0, 0, 0, 128), (1, 128, 0, 256), (2, 384, 0, 256),
            (3, 640, 0, 256), (4, 896, 0, 256), (5, 1152, 128, 128)]


def _bank_pieces(base, qlo, qn):
    """Split [base, base+qn) on 512 (PSUM bank) boundaries.
    Returns (piece_col, piece_qlo, piece_n)."""
    out = []
    col, q, remaining = base, qlo, qn
    while remaining:
        n = min(remaining, 512 - col % 512)
        out.append((col, q, n))
        col += n
        q += n
        remaining -= n
    return out


def _build_body(tc, aps, has_vbias):
    nc = tc.nc
    xcat_d, wqk_d, wv_d, bqk_d, maskcat_d, wvr_d, out_d = aps

    from contextlib import ExitStack
    ctx = ExitStack()
    sb = ctx.enter_context(tc.tile_pool(name="sb", bufs=1))
    xp = ctx.enter_context(tc.tile_pool(name="xp", bufs=2))
    ep = ctx.enter_context(tc.tile_pool(name="ep", bufs=6))
    otp = ctx.enter_context(tc.tile_pool(name="otp", bufs=4))
    rcp = ctx.enter_context(tc.tile_pool(name="rcp", bufs=4))
    psA = ctx.enter_context(tc.tile_pool(name="psA", bufs=2, space="PSUM"))
    psB = ctx.enter_context(tc.tile_pool(name="psB", bufs=2, space="PSUM"))

    # ---- persistent SBUF tensors (x stripe 0 + qk weights first: they
    # gate the first PE work; masks/wv are only needed later) ----
    xcs = [None] * NT

    def prefetch(nt, split=False):
        xc = xp.tile([128, KT2, 512], f8, tag="xc", name="xc")
        if split:
            nc.sync.dma_start(xc[:, 0:6, :],
                              xcat_d[:, 0:6, nt * 512:(nt + 1) * 512])
            nc.sync.dma_start(xc[:, 6:12, :],
                              xcat_d[:, 6:12, nt * 512:(nt + 1) * 512])
        else:
            nc.sync.dma_start(xc[:], xcat_d[:, :, nt * 512:(nt + 1) * 512])
        xcs[nt] = xc

    prefetch(0, split=True)
    # flat 2D weight transfers (contiguous >=512B runs avoid the DMA
    # read-modify-write latency penalty of small descriptor elements)
    wqk = sb.tile([128, KT2, 384], f8, tag="wqk")
    nc.sync.dma_start(wqk[:, 0:6, :].rearrange("p a b -> p (a b)"),
                      wqk_d[:, 0:6, :].rearrange("p a b -> p (a b)"))
    nc.sync.dma_start(wqk[:, 6:12, :].rearrange("p a b -> p (a b)"),
                      wqk_d[:, 6:12, :].rearrange("p a b -> p (a b)"))
    bqk = sb.tile([128, 3], f32, tag="bqk")
    nc.sync.dma_start(bqk[:], bqk_d[:])
    prefetch(1)
    wv = sb.tile([128, KT2, 192], f8, tag="wv")
    nc.sync.dma_start(wv[:].rearrange("p a b -> p (a b)"),
                      wv_d[:].rearrange("p a b -> p (a b)"))
    maskcat = sb.tile([128, 256], f16, tag="maskcat")
    nc.sync.dma_start(maskcat[:], maskcat_d[:])
    if has_vbias:
        wvr = sb.tile([1, 192], f16, tag="wvr")
        nc.sync.dma_start(wvr[:], wvr_d[:])
        ones1 = sb.tile([1, 128], f16, tag="ones1")
        nc.vector.memset(ones1[:], 1.0)

    qkT = sb.tile([128, 3 * S], f16, tag="qkT")    # q01 | k01 | q2k2
    k2sb = sb.tile([128, S], f16, tag="k2sb")      # rows 0:64 = head2 K
    vsb = sb.tile([128, 32 * VW], f16, tag="vsb")

    # static ones columns of vsb (denominator trick)
    ones_ap = vsb[:, 0:32 * VW].rearrange(
        "p (r h o) -> p r h o", r=32, h=HPC, o=65)[:, :, :, 64:65]
    nc.vector.memset(ones_ap, WSCALE)

    # ---- projection stripes (split into two halves so they interleave
    # between attention chunks and keep the PE queue from head-of-line
    # blocking the exp pipeline) ----
    def proj_group(nt, g):
        xc = xcs[nt]
        pg = psB.tile([128, 512], f32, tag="psB", name="pg")
        for i, (wp, xp_) in enumerate(PAIR_TERMS):
            nc.tensor.matmul(
                pg[:],
                wqk[:, 2 * wp:2 * wp + 2, 128 * g:128 * g + 128],
                xc[:, 2 * xp_:2 * xp_ + 2, :],
                start=(i == 0), stop=(i == len(PAIR_TERMS) - 1),
                perf_mode=DR,
            )
        nc.vector.tensor_scalar_add(
            qkT[:, g * S + nt * 512: g * S + nt * 512 + 512],
            pg[:], bqk[:, g:g + 1])
        if g == 2:
            # realign head-2 K rows 64:128 -> k2sb rows 0:64
            nc.sync.dma_start(
                k2sb[0:64, nt * 512:(nt + 1) * 512],
                qkT[64:128, 2 * S + nt * 512: 2 * S + nt * 512 + 512])

    def do_stripe_a(nt):
        proj_group(nt, 0)
        proj_group(nt, 1)

    def do_stripe_b(nt):
        proj_group(nt, 2)
        for rt4 in range(4):
            rt = nt * 4 + rt4
            xc = xcs[nt]
            pv = psB.tile([128, 512], f32, tag="psB", name="pv")
            for i, (wp, xp_) in enumerate(PAIR_TERMS):
                nc.tensor.matmul(
                    pv[:, 0:192],
                    xc[:, 2 * xp_:2 * xp_ + 2, rt4 * 128:rt4 * 128 + 128],
                    wv[:, 2 * wp:2 * wp + 2, :],
                    start=(i == 0),
                    stop=(False if has_vbias
                          else i == len(PAIR_TERMS) - 1),
                    perf_mode=DR,
                )
            if has_vbias:
                nc.tensor.matmul(pv[:, 0:192], ones1[:], wvr[:],
                                 start=False, stop=True)
            nc.vector.tensor_copy(
                vsb[:, rt * VW:(rt + 1) * VW].rearrange(
                    "p (h o) -> p h o", h=HPC, o=65)[:, :, 0:64],
                pv[:, 0:192].rearrange("p (h o) -> p h o", h=HPC, o=64))
        if nt + 2 < NT:
            prefetch(nt + 2)

    # ---- attention chunks ----
    def q_sl(hi, pos, n):
        if hi < 2:
            return qkT[64 * hi:64 * hi + 64, pos:pos + n]
        return qkT[0:64, 2 * S + pos: 2 * S + pos + n]

    def k_sl(hi, pos, n):
        if hi < 2:
            return qkT[64 * hi:64 * hi + 64, S + pos: S + pos + n]
        return k2sb[0:64, pos:pos + n]

    chunk_ets = {}

    def do_scores(c):
        blocks = _chunk_blocks(c)
        ncols = blocks[-1][1] + blocks[-1][3]
        pss = [None] * HPC
        ets = [None] * HPC
        chunk_ets[c] = ets

        def exp_mask(hi):
            et = ep.tile([128, 1408], f16, tag="et", name="et")
            if DBG_SPLIT_EXP:
                for b0 in range(0, ncols, 512):
                    n = min(512, ncols - b0)
                    nc.scalar.activation(
                        et[:, b0:b0 + n], pss[hi][:, b0:b0 + n],
                        mybir.ActivationFunctionType.Exp,
                        scale=0.125 / (WSCALE * WSCALE))
            else:
                nc.scalar.activation(et[:, 0:ncols], pss[hi][:, 0:ncols],
                                     mybir.ActivationFunctionType.Exp,
                                     scale=0.125 / (WSCALE * WSCALE))
            # triangle masks on diagonal blocks (pairs at stride 256).
            # lower-tri on the t0 block + t1's 2nd q-half; upper-tri on
            # t4's 1st q-half + the t5 block. c=0 starts at t2 so its
            # upper pair sits at cols 512/768; c=15 has no t4/t5.
            up_base = 512 if c == 0 else (None if c == C - 1 else 896)
            if DBG_PLAIN_MASK:
                if c != 0:
                    for c0 in (0, 256):
                        nc.vector.tensor_mul(et[:, c0:c0 + 128],
                                             et[:, c0:c0 + 128],
                                             maskcat[:, 0:128])
                if up_base is not None:
                    for c0 in (up_base, up_base + 256):
                        nc.vector.tensor_mul(et[:, c0:c0 + 128],
                                             et[:, c0:c0 + 128],
                                             maskcat[:, 128:256])
            else:
                if c != 0:
                    lo = et[:, 0:512].rearrange(
                        "p (a b) -> p a b", a=2, b=256)[:, :, 0:128]
                    nc.vector.tensor_mul(
                        lo, lo,
                        maskcat[:, 0:128].unsqueeze(1).broadcast_to(
                            [128, 2, 128]))
                if up_base is not None:
                    up = et[:, up_base:up_base + 512].rearrange(
                        "p (a b) -> p a b", a=2, b=256)[:, :, 0:128]
                    nc.vector.tensor_mul(
                        up, up,
                        maskcat[:, 128:256].unsqueeze(1).broadcast_to(
                            [128, 2, 128]))
            ets[hi] = et

        # scores for all heads, exp/mask trailing one head behind so the
        # PE never waits on ACT before it has queued independent work
        for hi in range(HPC):
            ps = psA.tile([128, 1536], f32, tag="psA", name="ps")
            pss[hi] = ps
            pieces = []
            for (t, base, qlo, qn) in blocks:
                kt_abs = 2 * (c - 1) + t
                for (col, q0, n) in _bank_pieces(base, qlo, qn):
                    pieces.append((col, n, kt_abs, q0))
            for i, (col, n, kt_abs, q0) in enumerate(pieces):
                bank = col // 512
                first = (i == 0) or (pieces[i - 1][0] // 512 != bank)
                last = (i == len(pieces) - 1) or \
                    (pieces[i + 1][0] // 512 != bank)
                nc.tensor.matmul(
                    ps[:, col:col + n],
                    k_sl(hi, kt_abs * 128, 128),
                    q_sl(hi, c * 256 + q0, n),
                    start=first, stop=last,
                )
            if hi >= 1:
                exp_mask(hi - 1)
        exp_mask(HPC - 1)

    def do_pv(c):
        blocks = _chunk_blocks(c)
        ets = chunk_ets.pop(c)
        # PV: 3 heads x 2 query halves accumulate into ONE PSUM bank
        # (group index g = 3*qh + hi at column 65*g)
        po = psB.tile([128, 512], f32, tag="psB", name="po")
        for hi in range(HPC):
            for qh in range(2):
                tlist = [(t, base, qlo) for (t, base, qlo, qn) in blocks
                         if (t <= 4 if qh == 0 else t >= 1)]
                for i, (t, base, qlo) in enumerate(tlist):
                    kt_abs = 2 * (c - 1) + t
                    col = base + 128 * qh - qlo
                    g = 3 * qh + hi
                    nc.tensor.matmul(
                        po[:, 65 * g:65 * g + 65],
                        ets[hi][:, col:col + 128],
                        vsb[:, kt_abs * VW + 65 * hi:
                            kt_abs * VW + 65 * hi + 65],
                        start=(hi == 0 and qh == 0 and i == 0),
                        stop=(hi == HPC - 1 and qh == 1
                              and i == len(tlist) - 1),
                    )
        # normalize + writeback: one reciprocal + one broadcast multiply
        po6 = po[:, 0:390].rearrange("p (g o) -> p g o", g=6, o=65)
        rec = rcp.tile([128, 6], f32, tag="rec", name="rec")
        nc.vector.reciprocal(rec[:, 0:6].unsqueeze(2), po6[:, :, 64:65])
        ot = otp.tile([128, 384], f16, tag="ot", name="ot")
        nc.vector.tensor_mul(
            ot[:, 0:384].rearrange("p (g o) -> p g o", g=6, o=64),
            po6[:, :, 0:64],
            rec[:, 0:6].unsqueeze(2).broadcast_to([128, 6, 64]))
        for qh in range(2):
            nc.sync.dma_start(
                out_d[c * 256 + qh * 128: c * 256 + qh * 128 + 128, :],
                ot[:, 192 * qh:192 * qh + 192])

    # ---- software-pipelined schedule: PV deferred one chunk; stripe
    # halves fill PE slack while ACT works through the exps ----
    do_stripe_a(0)
    do_stripe_b(0)
    do_scores(0)
    do_stripe_a(1)
    do_stripe_b(1)
    for c in range(1, C):
        do_scores(c)
        h = c - 1
        if h < 12:
            nt = 2 + h // 2
            (do_stripe_a if h % 2 == 0 else do_stripe_b)(nt)
        do_pv(c - 1)
    do_pv(C - 1)
    ctx.close()


def build_program(has_vbias=False):
    nc = bacc.Bacc("TRN2", target_bir_lowering=False, debug=False)
    xcat_d = nc.dram_tensor("xcat", [128, KT2, S], f8, kind="ExternalInput").ap()
    wqk_d = nc.dram_tensor("wqk", [128, KT2, 384], f8, kind="ExternalInput").ap()
    wv_d = nc.dram_tensor("wv", [128, KT2, 192], f8, kind="ExternalInput").ap()
    bqk_d = nc.dram_tensor("bqk", [128, 3], f32, kind="ExternalInput").ap()
    maskcat_d = nc.dram_tensor("maskcat", [128, 256], f16,
                               kind="ExternalInput").ap()
    wvr_d = nc.dram_tensor("wvr", [1, 192], f16, kind="ExternalInput").ap() \
        if has_vbias else None
    out_d = nc.dram_tensor("out", [S, 192], f16, kind="ExternalOutput").ap()
    with tile.TileContext(nc) as tc:
        _build_body(tc, (xcat_d, wqk_d, wv_d, bqk_d, maskcat_d, wvr_d, out_d),
                    has_vbias)
    nc.compile()
    return nc


def _fp8_pair(a):
    a8 = a.astype(FP8)
    r8 = (a - a8.astype(np.float32)).astype(FP8)
    return a8, r8


def _stack_kt(a8, ar8, ncols):
    # [768, ncols] fp8 pair -> [128, 12, ncols]
    lo = np.ascontiguousarray(a8.reshape(6, 128, ncols).transpose(1, 0, 2))
    hi = np.ascontiguousarray(ar8.reshape(6, 128, ncols).transpose(1, 0, 2))
    return np.ascontiguousarray(np.concatenate([lo, hi], axis=1))


def make_in_maps(hidden_states, Wq, bq, Wk, bk, Wv, bv):
    hs = np.asarray(hidden_states, np.float32)
    Wq = np.asarray(Wq, np.float32)
    Wk = np.asarray(Wk, np.float32)
    Wv = np.asarray(Wv, np.float32)
    bq = np.asarray(bq, np.float32)
    bk = np.asarray(bk, np.float32)
    bv = np.asarray(bv, np.float32)

    xcats = []
    for b in range(B):
        x8, xr8 = _fp8_pair(np.ascontiguousarray(hs[b].T))
        xcats.append(_stack_kt(x8, xr8, S))

    maskcat = np.ascontiguousarray(np.concatenate(
        [np.tril(np.ones((128, 128), np.float16)),
         np.triu(np.ones((128, 128), np.float16))], axis=1))

    has_vbias = bool(np.any(bv != 0.0))

    in_maps = []
    for core in range(N_CORES):
        h0 = HPC * (core % 4)
        wcols = np.concatenate(
            [Wq[:, h0 * 64:(h0 + 2) * 64], Wk[:, h0 * 64:(h0 + 2) * 64],
             Wq[:, (h0 + 2) * 64:(h0 + 3) * 64],
             Wk[:, (h0 + 2) * 64:(h0 + 3) * 64]], axis=1)
        w8, wr8 = _fp8_pair(np.float32(WSCALE) * wcols)
        wqkcat = _stack_kt(w8, wr8, 384)
        v8, vr8 = _fp8_pair(
            np.float32(WSCALE)
            * np.ascontiguousarray(Wv[:, h0 * 64:(h0 + 3) * 64]))
        wvcat = _stack_kt(v8, vr8, 192)
        # biases ride on the WSCALE-scaled q/k/v
        bqk = np.zeros((128, 3), np.float32)
        bqk[:, 0] = WSCALE * bq[h0 * 64:(h0 + 2) * 64]
        bqk[:, 1] = WSCALE * bk[h0 * 64:(h0 + 2) * 64]
        bqk[0:64, 2] = WSCALE * bq[(h0 + 2) * 64:(h0 + 3) * 64]
        bqk[64:128, 2] = WSCALE * bk[(h0 + 2) * 64:(h0 + 3) * 64]
        m = {
            "xcat": xcats[core // 4],
            "wqk": wqkcat,
            "wv": wvcat,
            "bqk": bqk,
            "maskcat": maskcat,
        }
        if has_vbias:
            m["wvr"] = np.ascontiguousarray(
                (WSCALE * bv[h0 * 64:(h0 + 3) * 64]).reshape(1, 192)
                .astype(np.float16))
        in_maps.append(m)
    return in_maps


_NC_CACHE = None
_NC_CACHE_FLAG = None


def kernel(hidden_states, Wq, bq, Wk, bk, Wv, bv):
    global _NC_CACHE, _NC_CACHE_FLAG
    has_vbias = bool(np.any(np.asarray(bv) != 0.0))
    if _NC_CACHE is None or _NC_CACHE_FLAG != has_vbias:
        _NC_CACHE = build_program(has_vbias)
        _NC_CACHE_FLAG = has_vbias
    nc = _NC_CACHE
    in_maps = make_in_maps(hidden_states, Wq, bq, Wk, bk, Wv, bv)
    res = bass_utils.run_bass_kernel_spmd(nc, in_maps,
                                          core_ids=list(range(N_CORES)))
    out = np.zeros((B, S, H * D), np.float32)
    for core in range(N_CORES):
        b = core // 4
        h0 = HPC * (core % 4)
        out[b, :, h0 * 64:(h0 + HPC) * 64] = \
            res.results[core]["out"].astype(np.float32)
    return out
